# revision 33
# baseline (speedup 1.0000x reference)
"""Trainium2 Bass kernel for CoordPE + message-passing GNN.

Sharding: 8 cores = 2 batches x 4 query-chunks of 512 rows each.

Per core, pair tensors live in [j=128-partition, (jt, i)-free] layout.
Key structure vs the naive version:
  - sq from ONE augmented 5-dim matmul (no DVE assembly), d = sqrt(sq+eps)
    straight from PSUM (no max0 pass).
  - the 16 RBF planes exp(-g(d-c_r)^2) are generated as 4 direct
    Derivative_Erf activations (erf'(x) = 2/sqrt(pi) * exp(-x^2)) plus 12
    bf16 DVE chain steps u_{a-1} = (k_a * tau) * u_a with
    tau = exp(-2*g*dc*d) (downward chains: no overflow, no underflow in
    the relevant range).
  - each plane is reduced over j by PE matmuls whose stationary operand is
    rbf_w[r,:] broadcast across partitions, accumulating h_geo^T directly
    in PSUM (mean, sqrt(pi)/2 and rbf_w fused; no separate rbf_local).
  - msg bias folded as rank-1 update msg += msg_b (x) rowsum(W).
  - MP layers AllGather X = h @ msg_w in bf16 (4x less traffic than f32 h).
"""
import math
import sys

import numpy as np

sys.path.insert(0, "/opt/trn_rl_repo")

B, L, D, K, R = 2, 2048, 128, 3, 16
NCORES = 8
CH = L // 4          # 512 queries per core
NJT = L // 128       # 16 j-tiles
FREE = NJT * CH      # 8192 free extent of pair tensors
EPS = 3e-4           # sqrt bias; also guards fp32 cancellation negatives


def build_program(gamma, centers, n_rep=1, use_cc=True, num_devices=NCORES,
                  debug_outs=False, loop_reps=0, ip_f32r=False,
                  sim_safe_act=False, ag_f32=False):
    import contextlib

    import concourse.tile as tile
    from concourse import bacc, mybir

    AF = mybir.ActivationFunctionType
    ALU = mybir.AluOpType
    dt = mybir.dt
    f32 = dt.float32
    bf16 = dt.bfloat16
    f16 = dt.float16

    gamma = float(gamma)
    centers = [float(c) for c in centers]
    delta = centers[1] - centers[0]
    sqg = math.sqrt(gamma)
    # chain: u_a = tau * u_{a+1} with tau = kbar * exp(-2*g*delta*d); the
    # per-step residual constants are folded into redW on the host (their
    # log stays within +-30, fine for bf16 planes and f32->bf16 weights).
    ln_kbar = gamma * delta * (centers[7] + centers[8])

    nc = bacc.Bacc("TRN2", target_bir_lowering=False, debug=False,
                   num_devices=num_devices)

    def dram_in(name, shape, dtype=f32):
        return nc.dram_tensor(name, shape, dtype, kind="ExternalInput")

    f32ip = dt.float32r if ip_f32r else f32
    cj_aug = dram_in("cj_aug", [5, L], f32ip)     # [x,y,z,|x|^2,1] (j side)
    ci_aug = dram_in("ci_aug", [5, CH], f32ip)    # [-2x,-2y,-2z,1,|x|^2]
    h_atomT = dram_in("h_atomT", [D, CH], bf16)
    out_w = dram_in("out_w", [D, 2 * D], bf16)    # [din, (half, dout)]
    self_w = dram_in("self_w", [D, K * D], bf16)
    msg_w = dram_in("msg_w", [D, K * D], bf16)
    upd_w = dram_in("upd_w", [D, K * 2 * D], bf16)
    redW = dram_in("redW", [128, R * D], bf16)    # rows = rbf_w[r]*spi/(2L)
    # cols: hg,out,self x3,upd x3, eps, -sqg*c_top x6, ln_kbar
    biases = dram_in("biases", [128, 16])
    msgb = dram_in("msgb", [1, K * D], bf16)
    ones_c = dram_in("ones_c", [128, 1], bf16)
    out_hT = nc.dram_tensor("out_hT", [D, CH], f32, kind="ExternalOutput")
    if debug_outs:
        dbg_d = nc.dram_tensor("dbg_d", [128, FREE], f32,
                               kind="ExternalOutput")
        dbg_hg = nc.dram_tensor("dbg_hg", [D, CH], f32,
                                kind="ExternalOutput")
        dbg_u = nc.dram_tensor("dbg_u", [128, FREE], f32,
                               kind="ExternalOutput")

    with tile.TileContext(nc) as tc:
        with (
            tc.tile_pool(name="const", bufs=1) as cpool,
            tc.tile_pool(name="dtens", bufs=1) as dpool_sb,
            tc.tile_pool(name="wtau", bufs=1) as wpool,
            tc.tile_pool(name="uplanes", bufs=2) as upool,
            tc.tile_pool(name="hmy", bufs=2) as hpool,
            tc.tile_pool(name="work", bufs=1) as work,
            tc.tile_pool(name="xbuf", bufs=2) as xpool,
            tc.tile_pool(name="psA", bufs=2, space="PSUM") as psA,
            tc.tile_pool(name="psHG", bufs=1, space="PSUM") as psHG,
            tc.tile_pool(name="psSW", bufs=1, space="PSUM") as psSW,
            tc.tile_pool(name="psX", bufs=1, space="PSUM") as psX,
            tc.tile_pool(name="psMM", bufs=2, space="PSUM") as psMM,
            tc.tile_pool(name="dram", bufs=1, space="DRAM") as dpool,
        ):
            def load(handle, shape, tag, dtype=f32):
                t = cpool.tile(shape, dtype, tag=tag)
                nc.sync.dma_start(t[:], handle.ap())
                return t

            t_cj = load(cj_aug, [5, L], "cj", f32ip)
            t_ci = load(ci_aug, [5, CH], "ci", f32ip)
            t_hat = load(h_atomT, [D, CH], "hat", bf16)
            t_outw = load(out_w, [D, 2 * D], "outw", bf16)
            t_selfw = load(self_w, [D, K * D], "selfw", bf16)
            t_msgw = load(msg_w, [D, K * D], "msgw", bf16)
            t_updw = load(upd_w, [D, K * 2 * D], "updw", bf16)
            t_redW = load(redW, [128, R * D], "redW", bf16)
            t_bias = load(biases, [128, 16], "bias")
            t_msgb = load(msgb, [1, K * D], "msgb", bf16)
            t_ones = load(ones_c, [128, 1], "ones", bf16)

            loop_cm = (tc.For_i(0, loop_reps, 1) if loop_reps
                       else contextlib.nullcontext())
            with loop_cm:
              for _rep in range(n_rep):
                # ---- P1: pairwise sq -> d ----
                d = dpool_sb.tile([128, FREE], f32, tag="d")
                for jt in range(NJT):
                    ip = psA.tile([128, CH], f32, tag="ip")
                    nc.tensor.matmul(ip[:], t_cj[:, jt * 128:(jt + 1) * 128],
                                     t_ci[:], start=True, stop=True)
                    nc.scalar.activation(d[:, jt * CH:(jt + 1) * CH], ip[:],
                                         AF.Sqrt, bias=t_bias[:, 8:9])
                if debug_outs:
                    nc.sync.dma_start(dbg_d.ap(), d[:])

                # ---- P2a: tau (chain multiplier, kbar folded into bias) ----
                tau = wpool.tile([128, FREE], bf16, tag="tau")
                nc.scalar.activation(tau[:], d[:], AF.Exp,
                                     scale=-2.0 * gamma * delta,
                                     bias=t_bias[:, 15:16])

                # ---- P3: 16 RBF planes, reduced into h_geo^T PSUM ----
                # groups: tops c2,c5,c8,c11,c13,c15; chains descend
                TOPS = [2, 5, 8, 11, 13, 15]
                CHAINS = [[1, 0], [4, 3], [7, 6], [10, 9], [12], [14]]
                n_planes = R
                hg_ps = psHG.tile([D, CH], f32, tag="hg")
                plane_no = [0]

                def reduce_plane(u, r):
                    first = plane_no[0] == 0
                    last = plane_no[0] == n_planes - 1
                    plane_no[0] += 1
                    for jt in range(NJT):
                        nc.tensor.matmul(
                            hg_ps[:], t_redW[:, r * D:(r + 1) * D],
                            u[:, jt * CH:(jt + 1) * CH],
                            start=(first and jt == 0),
                            stop=(last and jt == NJT - 1))

                anchor_fn = AF.Exp if sim_safe_act else AF.Derivative_Erf
                anchors = {}

                def emit_anchor(g):
                    u = upool.tile([128, FREE], bf16, tag="anc", bufs=4)
                    nc.scalar.activation(u[:], d[:], anchor_fn,
                                         scale=-1.0 if sim_safe_act else sqg,
                                         bias=t_bias[:, 9 + g:10 + g])
                    if debug_outs and g == 0:
                        nc.sync.dma_start(dbg_u.ap(), u[:])
                    reduce_plane(u, TOPS[g])
                    anchors[g] = u

                def emit_chains(g):
                    u = anchors[g]
                    for a in CHAINS[g]:
                        u2 = upool.tile([128, FREE], bf16, tag="u")
                        nc.vector.tensor_tensor(u2[:], tau[:], u[:],
                                                ALU.mult)
                        reduce_plane(u2, a)
                        u = u2

                for g in range(4):
                    emit_anchor(g)
                emit_chains(0)
                emit_anchor(4)
                emit_chains(1)
                emit_anchor(5)
                for g in (2, 3, 4, 5):
                    emit_chains(g)

                # ---- P2b: W + rowsum(W) ----
                w_pair = wpool.tile([128, FREE], bf16, tag="wp")
                nc.scalar.activation(w_pair[:], d[:], AF.Exp, scale=-1.0)
                sw_ps = psSW.tile([1, CH], f32, tag="sw")
                for jt in range(NJT):
                    nc.tensor.matmul(sw_ps[:], t_ones[:],
                                     w_pair[:, jt * CH:(jt + 1) * CH],
                                     start=(jt == 0), stop=(jt == NJT - 1))
                sw_sb = work.tile([1, CH], bf16, tag="swsb")
                nc.scalar.activation(sw_sb[:], sw_ps[:], AF.Identity)

                # ---- P4: h_geo bias, h0 ----
                hg = work.tile([D, CH], bf16, tag="hg")
                nc.scalar.activation(hg[:], hg_ps[:], AF.Identity,
                                     bias=t_bias[:, 0:1])
                if debug_outs:
                    nc.sync.dma_start(dbg_hg.ap(), hg[:])
                h0_ps = psMM.tile([D, CH], f32, tag="mm")
                nc.tensor.matmul(h0_ps[:], t_outw[:, 0:D], t_hat[:],
                                 start=True, stop=False)
                nc.tensor.matmul(h0_ps[:], t_outw[:, D:2 * D], hg[:],
                                 start=False, stop=True)
                h_my = hpool.tile([D, CH], bf16, tag="hmy")
                nc.scalar.activation(h_my[:], h0_ps[:], AF.Identity,
                                     bias=t_bias[:, 1:2])

                # ---- P5: MP layers ----
                for k in range(K):
                    # X_my = h_my @ msg_w[k]  -> [j-local, dout] bf16
                    x_ps = psX.tile([128, 4 * D], f32, tag="xps")
                    for c in range(4):
                        nc.tensor.matmul(
                            x_ps[:, c * D:(c + 1) * D],
                            h_my[:, c * 128:(c + 1) * 128],
                            t_msgw[:, k * D:(k + 1) * D],
                            start=True, stop=True)
                    x_sb = xpool.tile([128, NJT * D], bf16, tag="xsb", bufs=1)
                    if use_cc:
                        # f32 collective: 2-byte-dtype AllGathers wedge the
                        # device at scale (empirically); f32 [128,512] is the
                        # proven-stable shape. DMA straight from PSUM.
                        xmy = xpool.tile([128, 4 * D], f32, tag="xmy", bufs=1)
                        nc.vector.tensor_copy(xmy[:], x_ps[:])
                        ag_in = dpool.tile([128, 4 * D], f32, tag="agin")
                        ag_out = dpool.tile([4, 128, 4 * D], f32,
                                            tag="agout")
                        nc.sync.dma_start(ag_in[:], xmy[:])
                        nc.gpsimd.collective_compute(
                            "AllGather", ALU.bypass,
                            replica_groups=[[0, 1, 2, 3], [4, 5, 6, 7]],
                            ins=[ag_in.opt()], outs=[ag_out.opt()],
                        )
                        x_sb32 = xpool.tile([128, NJT * D], f32, tag="xsb32",
                                            bufs=1)
                        nc.sync.dma_start(
                            x_sb32[:].rearrange("p (q f) -> p q f", q=4),
                            ag_out[:].transpose([1, 0, 2]))
                        nc.vector.tensor_copy(x_sb[:], x_sb32[:])
                    else:
                        xmy = xpool.tile([128, 4 * D], bf16, tag="xmy", bufs=1)
                        nc.vector.tensor_copy(xmy[:], x_ps[:])
                        for q in range(4):
                            nc.vector.tensor_copy(
                                x_sb[:, q * 4 * D:(q + 1) * 4 * D], xmy[:])

                    self_ps = psMM.tile([D, CH], f32, tag="mm")
                    nc.tensor.matmul(self_ps[:],
                                     t_selfw[:, k * D:(k + 1) * D], h_my[:],
                                     start=True, stop=True)
                    self_sb = work.tile([D, CH], bf16, tag="selfsb")
                    nc.scalar.activation(self_sb[:], self_ps[:], AF.Identity,
                                         bias=t_bias[:, 2 + k:3 + k])

                    msg_ps = psMM.tile([D, CH], f32, tag="mm")
                    for jt in range(NJT):
                        nc.tensor.matmul(
                            msg_ps[:], x_sb[:, jt * D:(jt + 1) * D],
                            w_pair[:, jt * CH:(jt + 1) * CH],
                            start=(jt == 0), stop=False)
                    nc.tensor.matmul(msg_ps[:],
                                     t_msgb[:, k * D:(k + 1) * D], sw_sb[:],
                                     start=False, stop=True)
                    msg_sb = work.tile([D, CH], bf16, tag="msgsb")
                    nc.scalar.activation(msg_sb[:], msg_ps[:], AF.Identity)

                    upd_ps = psMM.tile([D, CH], f32, tag="mm")
                    nc.tensor.matmul(upd_ps[:],
                                     t_updw[:, (2 * k) * D:(2 * k + 1) * D],
                                     self_sb[:], start=True, stop=False)
                    nc.tensor.matmul(
                        upd_ps[:],
                        t_updw[:, (2 * k + 1) * D:(2 * k + 2) * D],
                        msg_sb[:], start=False, stop=True)
                    last = (k == K - 1)
                    h_my = hpool.tile([D, CH], f32 if last else bf16,
                                      tag="hout" if last else "hmy")
                    nc.scalar.activation(h_my[:], upd_ps[:], AF.Identity,
                                         bias=t_bias[:, 5 + k:6 + k])

                nc.sync.dma_start(out_hT.ap(), h_my[:])

    nc.compile()
    return nc


def make_in_maps(inputs):
    """Shard full inputs into per-core input maps (host side)."""
    coords = np.asarray(inputs["coords"], np.float32)        # [B, L, 3]
    Z = np.asarray(inputs["Z"])                              # [B, L]
    atom_emb = np.asarray(inputs["atom_emb"], np.float32)
    gamma = float(np.asarray(inputs["gamma"]))
    centers = np.asarray(inputs["rbf_centers"], np.float64)
    rbf_w = np.asarray(inputs["rbf_w"], np.float32)
    rbf_b = np.asarray(inputs["rbf_b"], np.float32)
    out_w = np.asarray(inputs["out_w"], np.float32)
    out_b = np.asarray(inputs["out_b"], np.float32)
    self_w = np.asarray(inputs["self_w"], np.float32)
    self_b = np.asarray(inputs["self_b"], np.float32)
    msg_w = np.asarray(inputs["msg_w"], np.float32)
    msg_b = np.asarray(inputs["msg_b"], np.float32)
    upd_w = np.asarray(inputs["upd_w"], np.float32)
    upd_b = np.asarray(inputs["upd_b"], np.float32)

    sqg = math.sqrt(gamma)
    ones = np.ones(D, np.float32)
    cen64 = centers.astype(np.float64)
    delta0 = float(cen64[1] - cen64[0])
    ln_kbar = float(gamma) * delta0 * float(cen64[7] + cen64[8])
    TOPS = [2, 5, 8, 11, 13, 15]
    bias_cols = np.stack(
        [rbf_b, out_b, self_b[0], self_b[1], self_b[2],
         upd_b[0], upd_b[1], upd_b[2], EPS * ones]
        + [-sqg * float(cen64[t]) * ones for t in TOPS]
        + [ln_kbar * ones],
        axis=1).astype(np.float32)                            # [128, 16]
    # planes carry 2/sqrt(pi); fold sqrt(pi)/2, 1/L and the chain residual
    # constants into the reduction weights
    gam = float(gamma)
    cen = centers.astype(np.float64)
    delta = float(cen[1] - cen[0])
    CHAINS = [[1, 0], [4, 3], [7, 6], [10, 9], [12], [14]]
    scale_r = np.zeros(R)
    for g, top in enumerate(TOPS):
        logp = 0.0
        scale_r[top] = 1.0
        for n, a in enumerate(CHAINS[g], start=1):
            logp += gam * delta * (cen[a] + cen[a + 1])
            # plane carries kbar^n / exp(logp) extra; invert it here
            scale_r[a] = math.exp(logp - n * ln_kbar)
    redW = np.zeros((128, R * D), np.float32)
    spi = math.sqrt(math.pi) / 2.0
    for r in range(R):
        redW[:, r * D:(r + 1) * D] = rbf_w[r][None, :] * (spi / L * scale_r[r])
    self_w_t = np.concatenate([self_w[k] for k in range(K)], axis=1)
    msg_w_t = np.concatenate([msg_w[k] for k in range(K)], axis=1)
    upd_w_t = np.concatenate(
        [upd_w[k, h * D:(h + 1) * D, :] for k in range(K) for h in range(2)],
        axis=1)                                               # [D, K*2*D]
    out_w_t = np.concatenate([out_w[0:D, :], out_w[D:2 * D, :]], axis=1)
    msgb = np.concatenate([msg_b[k][None, :] for k in range(K)], axis=1)

    import jax.numpy as jnp  # bfloat16 host arrays via jax/ml_dtypes

    def tobf16(x):
        return np.asarray(jnp.asarray(np.asarray(x, np.float32),
                                      dtype=jnp.bfloat16))

    in_maps = []
    for c in range(NCORES):
        b, q = divmod(c, 4)
        sl = slice(q * CH, (q + 1) * CH)
        cb = coords[b].astype(np.float64)                    # [L, 3]
        nrm = (cb ** 2).sum(-1)
        cj = np.zeros((5, L), np.float32)
        cj[0:3] = cb.T
        cj[3] = nrm
        cj[4] = 1.0
        ci = np.zeros((5, CH), np.float32)
        ci[0:3] = -2.0 * cb.T[:, sl]
        ci[3] = 1.0
        ci[4] = nrm[sl]
        in_maps.append({
            "cj_aug": cj,
            "ci_aug": ci,
            "h_atomT": tobf16(np.ascontiguousarray(atom_emb[Z[b, sl]].T)),
            "out_w": tobf16(out_w_t),
            "self_w": tobf16(self_w_t),
            "msg_w": tobf16(msg_w_t),
            "upd_w": tobf16(upd_w_t),
            "redW": tobf16(redW),
            "biases": np.ascontiguousarray(bias_cols),
            "msgb": tobf16(msgb),
            "ones_c": tobf16(np.ones((128, 1), np.float32)),
        })
    return gamma, centers, in_maps


def kernel(**inputs):
    from concourse.bass_utils import run_bass_kernel_spmd

    gamma, centers, in_maps = make_in_maps(inputs)
    nc = build_program(gamma, centers)
    res = run_bass_kernel_spmd(nc, in_maps, core_ids=list(range(NCORES)))
    out = np.zeros((B, L, D), np.float32)
    for c in range(NCORES):
        b, q = divmod(c, 4)
        out[b, q * CH:(q + 1) * CH, :] = res.results[c]["out_hT"].T
    return out
